# revision 66
# baseline (speedup 1.0000x reference)
"""Trainium2 Bass kernel for a dense transformer attention block.

Reference (per batch b of 4, seq S=2048, embed E=1024, H=16 heads, D=64):
    q/k/v = x @ W{q,k,v}.T + b,  split heads
    attn  = softmax(q k^T / sqrt(D)),  ctx = attn @ v
    out   = LN(ctx @ Wo.T + bo + x) * ln_g + ln_b

Sharding (8 cores, no collectives): core c handles batch b=c//2 and query rows
[1024*(c%2), 1024*(c%2+1)).  Each core computes K/V projections for its full
batch (duplicated with its pair core, ~25% extra FLOPs, zero comms), attention
for all 16 heads over its 1024 query rows, out-projection + residual + LN for
its rows.  Host reassembles the 8 row-shards.

Core layout strategy:
  - scores computed TRANSPOSED: S^T[k, q] with K^T stationary / Q^T moving, so
    exp(S^T) feeds the ctx matmul directly as the moving operand - no PE
    transposes anywhere in the kernel.
  - softmax exp is SPLIT ACROSS THREE ENGINES.  Wk is pre-scaled by 1/ln2 on
    the host so PSUM scores hold p = s/ln2 = 8*log2(exp(s/8)).  ACT computes
    table exp (scale=0.125*ln2) for some k-tiles; Pool and DVE compute the
    fp8e4m3 BIT PATTERN of 2^(p/8) directly as int8(p + 55.5) (one
    tensor_scalar_add with round-to-nearest convert; fp8 bits of 2^u are
    8u+56, so the linear mantissa interp gives exp to ~3% rms - below the
    fp8 quantization the ACT path pays anyway).  The int8 result is bitcast
    to fp8e4 and fed straight to the ctx matmul.
  - per k-tile group, both heads' scores live in one [P, 2, 512] PSUM tile
    (2 banks, ring of 3), and exp output lands in a per-chunk persistent
    pt[P, 2 heads, 16 kt, 512] fp8 tile so ALL ctx matmuls run as clean
    even-aligned DoubleRow pairs (0.5 cyc/row).
  - chunks run qc-MAJOR (all pairs at q-half 0, then q-half 1) so the first
    half of the out-proj/LN tail interleaves into the second sweep; ctx DR
    pairs of chunk i-1 interleave INTO chunk i's score stream (CTX_POPS);
    the LN rstd uses the 0x5f3759df rsqrt bit-trick + one Newton step on
    Pool (no ACT table swap mid-stream, no DVE reciprocal).
  - softmax denominator from a ones-column appended to V (stationary [V_h|1],
    M=65): PSUM row 64 accumulates sum_k exp.  The ones column holds 1/16 so
    the reciprocal is directly 16/sum (CTX_SCALE=16 keeps fp8 ctx out of the
    subnormal range; /16 folded into the out-proj epilogue).
  - ctx lands as ctx^T[e, q], exactly the stationary layout out-proj needs.
  - q/k/v/out projections and ctx run in fp8e4 with DoubleRow (0.5 cyc/row);
    PSUM accumulation is always fp32, residual + layernorm in fp32.  The
    attention branch contributes only ~0.8% of the output magnitude (the
    residual dominates), so fp8 path error dilutes ~128x in the final output.
  - scores+exp for chunk i overlap ctx for chunk i-1 (software pipeline), and
    the next pair's projection matmuls are interleaved into the attention
    stream so the in-order PE queue always has work.
"""

import sys

if "/opt/trn_rl_repo" not in sys.path:
    sys.path.insert(0, "/opt/trn_rl_repo")

import numpy as np
import ml_dtypes

B, S, E = 4, 2048, 1024
H, D = 16, 64
NQ = S // 2          # query rows per core
P = 128
ET = E // P          # 8 e-tiles
KT = S // P          # 16 k-tiles
W65 = D + 1          # V head width incl. ones column
VW = H * W65         # 1040
NCORES = 8
CTX_SCALE = 16.0     # keep fp8 ctx in normal range

INV_LN2 = 1.4426950408889634
EXP_SCALE = 0.125 / INV_LN2     # ACT exp scale after Wk prescale
MAGIC = 55.5                    # fp8e4m3 exponent bias*8 - 0.5 (round-nearest)

# per-chunk k-tile -> exp engine: A=ACT table exp, V=DVE bit-trick.
# (Pool/GPSIMD cannot access PSUM.)  The ACT/DVE split is phase-aware:
# during startup (ci=0) and the V-projection stretch (ci=1) ACT is loaded
# with PSUM casts while DVE would otherwise idle, so DVE takes nearly all
# exp groups there; the qc0 sweep carries projection casts on ACT (a=8);
# the qc1 sweep has no casts so ACT takes more (a=10).
_A_COUNT = {0: 2, 1: 0}


def _exp_assign(ci, g):
    a = _A_COUNT.get(ci, 8 if ci <= 8 else 10)
    return "A" if ((g + 1) * a) // KT > (g * a) // KT else "V"

FP8 = ml_dtypes.float8_e4m3

_cache = {}


def _build_nc(skip_affine=False, skip_bias=False):
    import concourse.bass as bass
    import concourse.tile as tile
    from concourse import bacc, mybir

    f8 = mybir.dt.float8e4
    i8 = mybir.dt.int8
    f32 = mybir.dt.float32
    DR = mybir.MatmulPerfMode.DoubleRow

    nc = bacc.Bacc(None, target_bir_lowering=False, debug=False)

    d_xkT = nc.dram_tensor("xkT", [E, S], f8, kind="ExternalInput")
    d_xqT = nc.dram_tensor("xqT", [E, NQ], f8, kind="ExternalInput")
    d_xq = nc.dram_tensor("xq", [NQ, E], f32, kind="ExternalInput")
    # wq/wk pre-shuffled on host to [pair, p, t, m] so each pair's slice DMAs
    # as contiguous 1KB runs instead of 128B strided rows
    d_wqR = nc.dram_tensor("wqR", [ET, P, ET, P], f8, kind="ExternalInput")
    d_wkR = nc.dram_tensor("wkR", [ET, P, ET, P], f8, kind="ExternalInput")
    d_wvT = nc.dram_tensor("wvT", [E, E], f8, kind="ExternalInput")
    d_woT = nc.dram_tensor("woT", [E, E], f8, kind="ExternalInput")
    if not skip_bias:
        d_bq = nc.dram_tensor("bq", [E], f32, kind="ExternalInput")
        d_bk = nc.dram_tensor("bk", [E], f32, kind="ExternalInput")
        d_bv = nc.dram_tensor("bv", [E], f32, kind="ExternalInput")
    d_lng = nc.dram_tensor("lng", [E], f32, kind="ExternalInput")
    d_lnb = nc.dram_tensor("lnb", [E], f32, kind="ExternalInput")
    d_out = nc.dram_tensor("out", [NQ, E], f32, kind="ExternalOutput")

    def bcast_ap(d):
        ap = d[:]
        return bass.AP(tensor=ap.tensor, offset=ap.offset, ap=[[0, P], [1, E]])

    from contextlib import ExitStack

    with tile.TileContext(nc) as tc, ExitStack() as ctx:
        persist = ctx.enter_context(tc.tile_pool(name="persist", bufs=1))
        wslice = ctx.enter_context(tc.tile_pool(name="wslice", bufs=2))
        # qc-major chunk order keeps every pair's q/k tiles alive
        qkpool = ctx.enter_context(tc.tile_pool(name="qkpool", bufs=ET))
        ppool = ctx.enter_context(tc.tile_pool(name="ppool", bufs=3))
        misc = ctx.enter_context(tc.tile_pool(name="misc", bufs=4))
        xqp = ctx.enter_context(tc.tile_pool(name="xqp", bufs=4))
        outp = ctx.enter_context(tc.tile_pool(name="outp", bufs=4))
        psum = ctx.enter_context(tc.tile_pool(name="psum", bufs=2, space="PSUM"))

        dma = nc.sync

        # ---- persistent tiles ----
        XK = persist.tile([P, ET, S], f8, tag="XK")       # x[b]^T, e-tiles on dim1
        XQ = persist.tile([P, ET, NQ], f8, tag="XQ")      # my query rows ^T
        WV = persist.tile([P, ET, E], f8, tag="WV")
        WO = persist.tile([P, ET, E], f8, tag="WO")
        VG = [persist.tile([P, 2, VW], f8, tag=f"vg{g}", name=f"vg{g}")
              for g in range(KT // 2)]
        CTG = [persist.tile([P, 2, NQ], f8, tag=f"ctg{t}", name=f"ctg{t}")
               for t in range(ET // 2)]
        if not skip_bias:
            bqs = persist.tile([P, ET], f32, tag="bqs")
            bks = persist.tile([P, ET], f32, tag="bks")
            bvb = persist.tile([P, E], f32, tag="bvb")
        if not skip_affine:
            lngb = persist.tile([P, E], f32, tag="lngb")
            lnbb = persist.tile([P, E], f32, tag="lnbb")
        epsb = persist.tile([P, 1], f32, tag="epsb")
        # rsqrt magic constant 0x5f3759df as int32, for the tail bit-trick
        rmagic = persist.tile([P, 1], mybir.dt.int32, tag="rmagic")

        # ---- input loads, ordered by first use ----
        def load_wslices(p):
            wq_sl = wslice.tile([P, ET, P], f8, tag="wqsl", name="wqsl")
            wk_sl = wslice.tile([P, ET, P], f8, tag="wksl", name="wksl")
            nc.gpsimd.dma_start(out=wq_sl, in_=d_wqR[p])
            nc.gpsimd.dma_start(out=wk_sl, in_=d_wkR[p])
            return wq_sl, wk_sl

        w0 = load_wslices(0)
        if not skip_bias:
            dma.dma_start(out=bqs, in_=d_bq[:].rearrange("(t p) -> p t", p=P))
            dma.dma_start(out=bks, in_=d_bk[:].rearrange("(t p) -> p t", p=P))
        nc.vector.memset(epsb, 1e-5)
        nc.gpsimd.memset(rmagic, 0x5F3759DF)
        # preload the exp ACT table while DMAs stream
        tdummy = misc.tile([1, 1], f32, tag="tdummy", name="tdummy")
        nc.scalar.activation(out=tdummy, in_=epsb[0:1, 0:1],
                             func=mybir.ActivationFunctionType.Exp)
        # chunked x loads, ordered so pair-0's first Q and K projections can
        # start as early as possible
        def load_xq_chunk(ch):
            csl = slice(ch * 512, (ch + 1) * 512)
            dma.dma_start(out=XQ[:, :, csl],
                          in_=d_xqT[:, csl].rearrange("(t p) k -> p t k", p=P))

        def load_xk_chunk(ch):
            csl = slice(ch * 512, (ch + 1) * 512)
            dma.dma_start(out=XK[:, :, csl],
                          in_=d_xkT[:, csl].rearrange("(t p) k -> p t k", p=P))

        load_xq_chunk(0)
        # first XK chunk rides the ACT engine's DMA queue, in parallel with
        # XQ chunk 0 on the sync queue
        nc.scalar.dma_start(
            out=XK[:, :, 0:512],
            in_=d_xkT[:, 0:512].rearrange("(t p) k -> p t k", p=P))
        load_xq_chunk(1)
        for ch in range(1, 4):
            load_xk_chunk(ch)
        dma.dma_start(out=WV, in_=d_wvT[:].rearrange("(t p) m -> p t m", p=P))
        if not skip_bias:
            dma.dma_start(out=bvb, in_=bcast_ap(d_bv))
        for g in range(KT // 2):
            v4 = VG[g].rearrange("p j (h w) -> p j h w", w=W65)
            nc.vector.memset(v4[:, :, :, D:W65], 1.0 / CTX_SCALE)
        if not skip_affine:
            dma.dma_start(out=lngb, in_=bcast_ap(d_lng))
            dma.dma_start(out=lnbb, in_=bcast_ap(d_lnb))
        dma.dma_start(out=WO, in_=d_woT[:].rearrange("(t p) m -> p t m", p=P))

        # ---- QK projection for one pair (DoubleRow over e-tile pairs) ----
        # PSUM->SBUF casts ride Pool to keep DVE/ACT free for exp work.
        def qk_emitters(p, wq_sl, wk_sl, qt, ktt):
            ems = []
            for ch in range(2):
                def eq(ch=ch):
                    csl = slice(ch * 512, (ch + 1) * 512)
                    ps = psum.tile([P, 512], f32, tag="acc", name="mmps")
                    for e2 in range(ET // 2):
                        nc.tensor.matmul(
                            ps, wq_sl[:, 2 * e2:2 * e2 + 2, :],
                            XQ[:, 2 * e2:2 * e2 + 2, csl],
                            start=(e2 == 0), stop=(e2 == ET // 2 - 1),
                            perf_mode=DR,
                        )
                    if skip_bias:
                        nc.scalar.activation(
                            out=qt[:, csl], in_=ps,
                            func=mybir.ActivationFunctionType.Identity)
                    else:
                        nc.vector.tensor_scalar_add(
                            out=qt[:, csl], in0=ps, scalar1=bqs[:, p:p + 1])
                ems.append(eq)
            for ch in range(4):
                def ek(ch=ch):
                    csl = slice(ch * 512, (ch + 1) * 512)
                    ps = psum.tile([P, 512], f32, tag="acc", name="mmps")
                    for e2 in range(ET // 2):
                        nc.tensor.matmul(
                            ps, wk_sl[:, 2 * e2:2 * e2 + 2, :],
                            XK[:, 2 * e2:2 * e2 + 2, csl],
                            start=(e2 == 0), stop=(e2 == ET // 2 - 1),
                            perf_mode=DR,
                        )
                    if skip_bias:
                        nc.scalar.activation(
                            out=ktt[:, csl], in_=ps,
                            func=mybir.ActivationFunctionType.Identity)
                    else:
                        nc.vector.tensor_scalar_add(
                            out=ktt[:, csl], in0=ps, scalar1=bks[:, p:p + 1])
                ems.append(ek)
            return ems

        def new_qk_tiles():
            qt = qkpool.tile([P, NQ], f8, tag="qtp", name="qtp")
            ktt = qkpool.tile([P, S], f8, tag="ktp", name="ktp")
            return qt, ktt

        def new_pt():
            return ppool.tile([P, 2, KT, 512], f8, tag="pt", name="pt")

        # ---- scores + exp/bit-trick for one k-tile, both heads ----
        def scores_one_group(qt, ktt, qc, g, pt, ci=0):
            qsl = slice(qc * 512, (qc + 1) * 512)
            ksl = slice(g * P, (g + 1) * P)
            sps = psum.tile([P, 2, 512], f32, tag="spsum", name="sps", bufs=3)
            for h in range(2):
                hsl = slice(h * D, (h + 1) * D)
                nc.tensor.matmul(
                    sps[:, h, :], ktt[hsl, ksl], qt[hsl, qsl],
                    start=True, stop=True,
                )
            out = pt[:, :, g, :]
            eng = _exp_assign(ci, g)
            if eng == "A":
                nc.scalar.activation(
                    out=out, in_=sps,
                    func=mybir.ActivationFunctionType.Exp,
                    scale=EXP_SCALE,
                )
            else:
                nc.vector.tensor_scalar_add(
                    out=out.bitcast(i8), in0=sps, scalar1=MAGIC)

        # ctx closures interleave into the NEXT chunk's scores: 2 DR pairs per
        # group slot (plus the norm chain), so the PE never emits ctx as a
        # blocking block and the exp engines keep a full sps backlog.
        CTX_POPS = [2, 2, 2, 2, 3, 2, 2, 2, 1, 0, 0, 0, 0, 0, 0, 0]

        def scores_exp(qt, ktt, qc, pt, ci=0, interleave=None, ctx_ems=None):
            for g in range(KT):
                scores_one_group(qt, ktt, qc, g, pt, ci=ci)
                if ctx_ems:
                    for _ in range(CTX_POPS[g]):
                        if ctx_ems:
                            ctx_ems.pop(0)()
                if interleave and g >= 10:
                    interleave.pop(0)()
            while ctx_ems:
                ctx_ems.pop(0)()

        # ---- ctx + normalize emitters for a previously exp'd chunk ----
        def ctx_emitters(p, qc, pt):
            qsl = slice(qc * 512, (qc + 1) * 512)
            cps = [None, None]
            ems = []
            for h in range(2):
                hh = 2 * p + h
                vsl = slice(hh * W65, (hh + 1) * W65)
                for j in range(KT // 2):
                    def epair(h=h, j=j, vsl=vsl):
                        if j == 0:
                            cps[h] = psum.tile([65, 512], f32, tag="acc",
                                               name=f"cps{h}")
                        nc.tensor.matmul(
                            cps[h], VG[j][:, 0:2, vsl],
                            pt[:, h, 2 * j:2 * j + 2, :],
                            start=(j == 0), stop=(j == KT // 2 - 1),
                            perf_mode=DR,
                        )
                    ems.append(epair)

                def enorm(h=h, hh=hh):
                    recip = misc.tile([1, 512], f32, tag="recip", name="recip",
                                      bufs=2)
                    nc.vector.reciprocal(out=recip, in_=cps[h][64:65, :])
                    bc = misc.tile([D, 512], f32, tag="bc", name="bc", bufs=2)
                    nc.gpsimd.partition_broadcast(out_ap=bc, in_ap=recip)
                    r0 = (hh % 2) * D
                    nc.vector.tensor_mul(
                        out=CTG[hh // 4][r0:r0 + D, (hh // 2) % 2, qsl],
                        in0=cps[h][0:D, :], in1=bc,
                    )
                ems.append(enorm)
            return ems

        def ctx_norm(p, qc, pt):
            for em in ctx_emitters(p, qc, pt):
                em()

        # ---- pair-0 projection with scores/exp interleaved per K chunk ----
        qk0 = new_qk_tiles()
        ems0 = qk_emitters(0, *w0, *qk0)
        ems0[0]()  # Q proj chunk 0
        ems0[1]()  # Q proj chunk 1
        pt00 = new_pt()
        for ch in range(4):
            ems0[2 + ch]()  # K proj chunk ch covers k-tiles 4ch..4ch+3
            for g in range(4 * ch, 4 * ch + 4):
                scores_one_group(*qk0, 0, g, pt00, ci=0)

        # ---- V projection (DoubleRow); pair-0 qc1 exps and pair-1 projection
        # both interleave under it so the exp engines never starve ----
        w1 = load_wslices(1)
        qk1 = new_qk_tiles()
        ems1 = qk_emitters(1, *w1, *qk1)
        pt01 = new_pt()
        gi = 0
        for g in range(KT // 2):
            for j in range(2):
                kt = 2 * g + j
                ksl = slice(kt * P, (kt + 1) * P)
                for ch in range(2):
                    csl = slice(ch * 512, (ch + 1) * 512)
                    ps = psum.tile([P, 512], f32, tag="acc", name="mmps")
                    for e2 in range(ET // 2):
                        nc.tensor.matmul(
                            ps, XK[:, 2 * e2:2 * e2 + 2, ksl],
                            WV[:, 2 * e2:2 * e2 + 2, csl],
                            start=(e2 == 0), stop=(e2 == ET // 2 - 1),
                            perf_mode=DR,
                        )
                    v4 = VG[g].rearrange("p j (h w) -> p j h w", w=W65)
                    if skip_bias:
                        nc.scalar.activation(
                            out=v4[:, j, ch * 8:(ch + 1) * 8, 0:D],
                            in_=ps.rearrange("p (h w) -> p h w", w=D),
                            func=mybir.ActivationFunctionType.Identity)
                    else:
                        nc.vector.tensor_add(
                            out=v4[:, j, ch * 8:(ch + 1) * 8, 0:D],
                            in0=ps.rearrange("p (h w) -> p h w", w=D),
                            in1=bvb[:, csl].rearrange("p (h w) -> p h w", w=D),
                        )
            # keep the exp engines fed during the V stretch with pair-0 qc1
            for _ in range(2):
                if gi < KT:
                    scores_one_group(*qk0, 1, gi, pt01, ci=1)
                    gi += 1
            if g >= 1 and ems1:
                ems1.pop(0)()

        # ---- pair-0 qc0 ctx, remaining pair-1 projection ----
        ctx_norm(0, 0, pt00)
        for em in ems1:
            em()

        # ---- out projection (DoubleRow) + residual + layernorm ----
        # residual DMA (d_xq already carries +bo, folded on host); LN stats via
        # fused accum_out sums (mean) + an ACT Square pass (E[x^2]) - no
        # bn_stats pass.  The elementwise normalize runs on Pool for the
        # mid-stream tiles (ACT/DVE are busy with exp) and on DVE at drain.
        xq_sbs = [None] * ET

        def load_xq(nt):
            xq_sb = xqp.tile([P, E], f32, tag="xqt", name="xqt")
            dma.dma_start(out=xq_sb, in_=d_xq[nt * P:(nt + 1) * P, :])
            xq_sbs[nt] = xq_sb

        for nt in range(4):
            load_xq(nt)

        def tail_nt(nt, drain=False):
            # prefetch the +4 residual tile; emitting it here (not upfront)
            # keeps the 4-buf ring's reuse dep behind this tile's reads
            if nt + 4 < ET:
                load_xq(nt + 4)
            nsl = slice(nt * P, (nt + 1) * P)
            xq_sb = xq_sbs[nt]
            out_sb = outp.tile([P, E], f32, tag="outsb", name="outsb")
            scratch = outp.tile([P, E], f32, tag="sq", name="sq", bufs=2)
            sq2 = misc.tile([P, 2], f32, tag="sq2", name="sq2")
            stat = misc.tile([P, 4], f32, tag="stat", name="stat")
            rsum = misc.tile([P, 2], f32, tag="rsum", name="rsum")
            for ec in range(2):
                csl = slice(ec * 512, (ec + 1) * 512)
                ps = psum.tile([P, 512], f32, tag="acc", name="mmps")
                for t in range(ET // 2):
                    nc.tensor.matmul(
                        ps, CTG[t][:, 0:2, nsl], WO[:, 2 * t:2 * t + 2, csl],
                        start=(t == 0), stop=(t == ET // 2 - 1),
                        perf_mode=DR,
                    )
                # out = ps/CTX_SCALE + (x + bo), accumulating row sums
                nc.vector.scalar_tensor_tensor(
                    out=out_sb[:, csl], in0=ps, scalar=1.0 / CTX_SCALE,
                    in1=xq_sb[:, csl],
                    op0=mybir.AluOpType.mult, op1=mybir.AluOpType.add,
                    accum_out=rsum[:, ec:ec + 1],
                )
                nc.scalar.activation(
                    out=scratch[:, csl], in_=out_sb[:, csl],
                    func=mybir.ActivationFunctionType.Square,
                    accum_out=sq2[:, ec:ec + 1])
            nc.gpsimd.tensor_add(out=stat[:, 0:1], in0=rsum[:, 0:1],
                                 in1=rsum[:, 1:2])
            # var*E = (sq0+sq1) - (r0+r1)^2/E; the final /E folds into Sqrt's
            # scale.  Small stats run on Pool (SBUF-only; no stt there).
            nc.gpsimd.tensor_tensor(
                out=stat[:, 1:2], in0=stat[:, 0:1], in1=stat[:, 0:1],
                op=mybir.AluOpType.mult)                             # msum^2
            nc.gpsimd.tensor_scalar(
                out=stat[:, 1:2], in0=stat[:, 1:2], scalar1=1.0 / E,
                scalar2=None, op0=mybir.AluOpType.mult)              # msum^2/E
            nc.gpsimd.tensor_add(out=stat[:, 2:3], in0=sq2[:, 0:1], in1=sq2[:, 1:2])
            nc.gpsimd.tensor_sub(out=stat[:, 3:4], in0=stat[:, 2:3], in1=stat[:, 1:2])
            # rstd via the 0x5f3759df rsqrt bit-trick + one Newton step, all
            # on Pool: avoids an ACT Sqrt (whose table set would thrash the
            # exp table mid-stream) and a DVE reciprocal.
            var = misc.tile([P, 1], f32, tag="var", name="var")
            nc.gpsimd.tensor_scalar(
                out=var, in0=stat[:, 3:4], scalar1=1.0 / E, scalar2=1e-5,
                op0=mybir.AluOpType.mult, op1=mybir.AluOpType.add)   # var+eps
            std = misc.tile([P, 1], f32, tag="std", name="std")
            istd = std.bitcast(mybir.dt.int32)
            nc.vector.tensor_scalar(
                out=istd, in0=var.bitcast(mybir.dt.int32), scalar1=1,
                scalar2=None, op0=mybir.AluOpType.arith_shift_right)
            nc.gpsimd.tensor_tensor(
                out=istd, in0=rmagic, in1=istd, op=mybir.AluOpType.subtract)
            # Newton: y *= 1.5 - 0.5*var*y^2
            nwt = misc.tile([P, 1], f32, tag="nwt", name="nwt")
            nc.gpsimd.tensor_tensor(out=nwt, in0=std, in1=std,
                                    op=mybir.AluOpType.mult)
            nc.gpsimd.tensor_tensor(out=nwt, in0=nwt, in1=var,
                                    op=mybir.AluOpType.mult)
            nc.gpsimd.tensor_scalar(
                out=nwt, in0=nwt, scalar1=-0.5, scalar2=1.5,
                op0=mybir.AluOpType.mult, op1=mybir.AluOpType.add)
            nc.gpsimd.tensor_tensor(out=std, in0=std, in1=nwt,
                                    op=mybir.AluOpType.mult)          # rstd
            negmr = misc.tile([P, 1], f32, tag="negmr", name="negmr")
            nc.gpsimd.tensor_scalar(
                out=negmr, in0=stat[:, 0:1], scalar1=-1.0 / E,
                scalar2=None, op0=mybir.AluOpType.mult)
            nc.gpsimd.tensor_tensor(
                out=negmr, in0=negmr, in1=std, op=mybir.AluOpType.mult)  # -mean*rstd
            # (x - mu) * rstd == x*rstd + (-mu*rstd)
            norm_eng = nc.vector if drain else nc.gpsimd
            norm_eng.tensor_scalar(
                out=out_sb, in0=out_sb,
                scalar1=std[:, 0:1], scalar2=negmr[:, 0:1],
                op0=mybir.AluOpType.mult, op1=mybir.AluOpType.add,
            )
            if not skip_affine:
                nc.vector.tensor_mul(out=out_sb, in0=out_sb, in1=lngb)
                nc.gpsimd.tensor_add(out=out_sb, in0=out_sb, in1=lnbb)
            dma.dma_start(out=d_out[nsl, :], in_=out_sb)

        # ---- pipelined attention, qc-MAJOR: sweep qc=0 over pairs 1..7
        # (staging each next pair's projections), so all qc=0 ctx completes
        # mid-kernel; the qc=1 sweep then interleaves tail_nt(0..3).  ctx for
        # chunk i runs one chunk behind its scores/exp. ----
        qks = [None, qk1] + [None] * (ET - 2)
        ctxq = [(0, 1, pt01)]   # (0,0) already ctx'd above
        tail_q = []
        ci = 2
        for sweep_qc in (0, 1):
            for p in range(1, ET):
                pending = []
                if sweep_qc == 0 and p + 1 < ET:
                    wnxt = load_wslices(p + 1)
                    qks[p + 1] = new_qk_tiles()
                    pending = qk_emitters(p + 1, *wnxt, *qks[p + 1])
                ptc = new_pt()
                done = ctxq.pop(0)
                cems = ctx_emitters(*done)
                scores_exp(*qks[p], sweep_qc, ptc, ci=ci, interleave=pending,
                           ctx_ems=cems)
                if done[:2] == (ET - 1, 0):
                    tail_q = list(range(4))
                for em in pending:
                    em()
                if tail_q and not pending:
                    tail_nt(tail_q.pop(0))
                ctxq.append((p, sweep_qc, ptc))
                ci += 1

        ctx_norm(*ctxq.pop(0))
        for nt in tail_q + list(range(4, ET)):
            tail_nt(nt, drain=True)

    nc.compile()
    return nc


def _get_nc(skip_affine=False, skip_bias=False):
    key = ("nc", skip_affine, skip_bias)
    if key not in _cache:
        _cache[key] = _build_nc(skip_affine, skip_bias)
    return _cache[key]


def kernel(x, Wq, bq, Wk, bk, Wv, bv, Wo, bo, ln_g, ln_b, _trace=False, _tmpdir=None):
    from concourse.bass_utils import run_bass_kernel_spmd

    x = np.asarray(x, np.float32)

    def shuffle_w(W, scale=1.0):
        # W.T [e_in, e_out] -> [pair, p, t, m] with e_in = t*128+p, e_out of pair
        wT = (np.asarray(W, np.float32) * scale).T.reshape(ET, P, ET, P)
        return np.ascontiguousarray(wT.transpose(2, 1, 0, 3)).astype(FP8)

    wqR = shuffle_w(Wq)
    # Wk pre-scaled by 1/ln2: PSUM scores become s/ln2, which both the ACT
    # exp (scale=0.125*ln2) and the fp8-bits trick (int8(p + 55.5)) consume.
    wkR = shuffle_w(Wk, scale=INV_LN2)
    wvT = np.ascontiguousarray(np.asarray(Wv, np.float32).T).astype(FP8)
    woT = np.ascontiguousarray(np.asarray(Wo, np.float32).T).astype(FP8)
    vecs = {
        "bq": np.asarray(bq, np.float32),
        "bk": np.asarray(bk, np.float32) * np.float32(INV_LN2),
        "bv": np.asarray(bv, np.float32),
        "lng": np.asarray(ln_g, np.float32), "lnb": np.asarray(ln_b, np.float32),
    }
    bo_f = np.asarray(bo, np.float32)

    in_maps = []
    for c in range(NCORES):
        b, half = c // 2, c % 2
        xbT = np.ascontiguousarray(x[b].T).astype(FP8)
        xq_res = x[b, half * NQ:(half + 1) * NQ, :] + bo_f
        in_maps.append({
            "xkT": xbT,
            "xqT": np.ascontiguousarray(xbT[:, half * NQ:(half + 1) * NQ]),
            # residual rows with the out-proj bias pre-added (saves a DVE pass)
            "xq": xq_res,
            "wqR": wqR, "wkR": wkR, "wvT": wvT, "woT": woT,
            **vecs,
        })

    # ln_g == 1 / ln_b == 0 make the LN affine step an exact no-op; build the
    # specialized kernel for that case (general path kept as fallback)
    skip_affine = bool(
        np.all(np.asarray(ln_g) == 1.0) and np.all(np.asarray(ln_b) == 0.0))
    skip_bias = bool(
        np.all(np.asarray(bq) == 0.0) and np.all(np.asarray(bk) == 0.0)
        and np.all(np.asarray(bv) == 0.0))
    if skip_bias:
        for m in in_maps:
            del m["bq"], m["bk"], m["bv"]
    nc = _get_nc(skip_affine, skip_bias)
    _cache["last_nc"] = nc
    res = run_bass_kernel_spmd(
        nc, in_maps, list(range(NCORES)), trace=_trace, tmpdir=_tmpdir
    )
    out = np.empty((B, S, E), np.float32)
    for c in range(NCORES):
        b, half = c // 2, c % 2
        out[b, half * NQ:(half + 1) * NQ, :] = res.results[c]["out"]
    if _trace:
        _cache["last_result"] = res
    return out


# revision 67
# speedup vs baseline: 1.0005x; 1.0005x over previous
"""Trainium2 Bass kernel for a dense transformer attention block.

Reference (per batch b of 4, seq S=2048, embed E=1024, H=16 heads, D=64):
    q/k/v = x @ W{q,k,v}.T + b,  split heads
    attn  = softmax(q k^T / sqrt(D)),  ctx = attn @ v
    out   = LN(ctx @ Wo.T + bo + x) * ln_g + ln_b

Sharding (8 cores, no collectives): core c handles batch b=c//2 and query rows
[1024*(c%2), 1024*(c%2+1)).  Each core computes K/V projections for its full
batch (duplicated with its pair core, ~25% extra FLOPs, zero comms), attention
for all 16 heads over its 1024 query rows, out-projection + residual + LN for
its rows.  Host reassembles the 8 row-shards.

Core layout strategy:
  - scores computed TRANSPOSED: S^T[k, q] with K^T stationary / Q^T moving, so
    exp(S^T) feeds the ctx matmul directly as the moving operand - no PE
    transposes anywhere in the kernel.
  - softmax exp is SPLIT ACROSS THREE ENGINES.  Wk is pre-scaled by 1/ln2 on
    the host so PSUM scores hold p = s/ln2 = 8*log2(exp(s/8)).  ACT computes
    table exp (scale=0.125*ln2) for some k-tiles; Pool and DVE compute the
    fp8e4m3 BIT PATTERN of 2^(p/8) directly as int8(p + 55.5) (one
    tensor_scalar_add with round-to-nearest convert; fp8 bits of 2^u are
    8u+56, so the linear mantissa interp gives exp to ~3% rms - below the
    fp8 quantization the ACT path pays anyway).  The int8 result is bitcast
    to fp8e4 and fed straight to the ctx matmul.
  - per k-tile group, both heads' scores live in one [P, 2, 512] PSUM tile
    (2 banks, ring of 3), and exp output lands in a per-chunk persistent
    pt[P, 2 heads, 16 kt, 512] fp8 tile so ALL ctx matmuls run as clean
    even-aligned DoubleRow pairs (0.5 cyc/row).
  - chunks run qc-MAJOR (all pairs at q-half 0, then q-half 1) so the first
    half of the out-proj/LN tail interleaves into the second sweep; ctx DR
    pairs of chunk i-1 interleave INTO chunk i's score stream (CTX_POPS);
    the LN rstd uses the 0x5f3759df rsqrt bit-trick + one Newton step on
    Pool (no ACT table swap mid-stream, no DVE reciprocal).
  - softmax denominator from a ones-column appended to V (stationary [V_h|1],
    M=65): PSUM row 64 accumulates sum_k exp.  The ones column holds 1/16 so
    the reciprocal is directly 16/sum (CTX_SCALE=16 keeps fp8 ctx out of the
    subnormal range; /16 folded into the out-proj epilogue).
  - ctx lands as ctx^T[e, q], exactly the stationary layout out-proj needs.
  - q/k/v/out projections and ctx run in fp8e4 with DoubleRow (0.5 cyc/row);
    PSUM accumulation is always fp32, residual + layernorm in fp32.  The
    attention branch contributes only ~0.8% of the output magnitude (the
    residual dominates), so fp8 path error dilutes ~128x in the final output.
  - scores+exp for chunk i overlap ctx for chunk i-1 (software pipeline), and
    the next pair's projection matmuls are interleaved into the attention
    stream so the in-order PE queue always has work.
"""

import sys

if "/opt/trn_rl_repo" not in sys.path:
    sys.path.insert(0, "/opt/trn_rl_repo")

import numpy as np
import ml_dtypes

B, S, E = 4, 2048, 1024
H, D = 16, 64
NQ = S // 2          # query rows per core
P = 128
ET = E // P          # 8 e-tiles
KT = S // P          # 16 k-tiles
W65 = D + 1          # V head width incl. ones column
VW = H * W65         # 1040
NCORES = 8
CTX_SCALE = 16.0     # keep fp8 ctx in normal range

INV_LN2 = 1.4426950408889634
EXP_SCALE = 0.125 / INV_LN2     # ACT exp scale after Wk prescale
MAGIC = 55.5                    # fp8e4m3 exponent bias*8 - 0.5 (round-nearest)

# per-chunk k-tile -> exp engine: A=ACT table exp, V=DVE bit-trick.
# (Pool/GPSIMD cannot access PSUM.)  The ACT/DVE split is phase-aware:
# during startup (ci=0) and the V-projection stretch (ci=1) ACT is loaded
# with PSUM casts while DVE would otherwise idle, so DVE takes nearly all
# exp groups there; the qc0 sweep carries projection casts on ACT (a=8);
# the qc1 sweep has no casts so ACT takes more (a=10).
_A_COUNT = {0: 6, 1: 2}


def _exp_assign(ci, g):
    a = _A_COUNT.get(ci, 8 if ci <= 8 else 10)
    return "A" if ((g + 1) * a) // KT > (g * a) // KT else "V"

FP8 = ml_dtypes.float8_e4m3

_cache = {}


def _build_nc(skip_affine=False, skip_bias=False):
    import concourse.bass as bass
    import concourse.tile as tile
    from concourse import bacc, mybir

    f8 = mybir.dt.float8e4
    i8 = mybir.dt.int8
    f32 = mybir.dt.float32
    DR = mybir.MatmulPerfMode.DoubleRow

    nc = bacc.Bacc(None, target_bir_lowering=False, debug=False)

    d_xkT = nc.dram_tensor("xkT", [E, S], f8, kind="ExternalInput")
    d_xqT = nc.dram_tensor("xqT", [E, NQ], f8, kind="ExternalInput")
    d_xq = nc.dram_tensor("xq", [NQ, E], f32, kind="ExternalInput")
    # wq/wk pre-shuffled on host to [pair, p, t, m] so each pair's slice DMAs
    # as contiguous 1KB runs instead of 128B strided rows
    d_wqR = nc.dram_tensor("wqR", [ET, P, ET, P], f8, kind="ExternalInput")
    d_wkR = nc.dram_tensor("wkR", [ET, P, ET, P], f8, kind="ExternalInput")
    d_wvT = nc.dram_tensor("wvT", [E, E], f8, kind="ExternalInput")
    d_woT = nc.dram_tensor("woT", [E, E], f8, kind="ExternalInput")
    if not skip_bias:
        d_bq = nc.dram_tensor("bq", [E], f32, kind="ExternalInput")
        d_bk = nc.dram_tensor("bk", [E], f32, kind="ExternalInput")
        d_bv = nc.dram_tensor("bv", [E], f32, kind="ExternalInput")
    d_lng = nc.dram_tensor("lng", [E], f32, kind="ExternalInput")
    d_lnb = nc.dram_tensor("lnb", [E], f32, kind="ExternalInput")
    d_out = nc.dram_tensor("out", [NQ, E], f32, kind="ExternalOutput")

    def bcast_ap(d):
        ap = d[:]
        return bass.AP(tensor=ap.tensor, offset=ap.offset, ap=[[0, P], [1, E]])

    from contextlib import ExitStack

    with tile.TileContext(nc) as tc, ExitStack() as ctx:
        persist = ctx.enter_context(tc.tile_pool(name="persist", bufs=1))
        wslice = ctx.enter_context(tc.tile_pool(name="wslice", bufs=2))
        # qc-major chunk order keeps every pair's q/k tiles alive
        qkpool = ctx.enter_context(tc.tile_pool(name="qkpool", bufs=ET))
        ppool = ctx.enter_context(tc.tile_pool(name="ppool", bufs=3))
        misc = ctx.enter_context(tc.tile_pool(name="misc", bufs=4))
        xqp = ctx.enter_context(tc.tile_pool(name="xqp", bufs=4))
        outp = ctx.enter_context(tc.tile_pool(name="outp", bufs=4))
        psum = ctx.enter_context(tc.tile_pool(name="psum", bufs=2, space="PSUM"))

        dma = nc.sync

        # ---- persistent tiles ----
        XK = persist.tile([P, ET, S], f8, tag="XK")       # x[b]^T, e-tiles on dim1
        XQ = persist.tile([P, ET, NQ], f8, tag="XQ")      # my query rows ^T
        WV = persist.tile([P, ET, E], f8, tag="WV")
        WO = persist.tile([P, ET, E], f8, tag="WO")
        VG = [persist.tile([P, 2, VW], f8, tag=f"vg{g}", name=f"vg{g}")
              for g in range(KT // 2)]
        CTG = [persist.tile([P, 2, NQ], f8, tag=f"ctg{t}", name=f"ctg{t}")
               for t in range(ET // 2)]
        if not skip_bias:
            bqs = persist.tile([P, ET], f32, tag="bqs")
            bks = persist.tile([P, ET], f32, tag="bks")
            bvb = persist.tile([P, E], f32, tag="bvb")
        if not skip_affine:
            lngb = persist.tile([P, E], f32, tag="lngb")
            lnbb = persist.tile([P, E], f32, tag="lnbb")
        epsb = persist.tile([P, 1], f32, tag="epsb")
        # rsqrt magic constant 0x5f3759df as int32, for the tail bit-trick
        rmagic = persist.tile([P, 1], mybir.dt.int32, tag="rmagic")

        # ---- input loads, ordered by first use ----
        def load_wslices(p):
            wq_sl = wslice.tile([P, ET, P], f8, tag="wqsl", name="wqsl")
            wk_sl = wslice.tile([P, ET, P], f8, tag="wksl", name="wksl")
            nc.gpsimd.dma_start(out=wq_sl, in_=d_wqR[p])
            nc.gpsimd.dma_start(out=wk_sl, in_=d_wkR[p])
            return wq_sl, wk_sl

        w0 = load_wslices(0)
        if not skip_bias:
            dma.dma_start(out=bqs, in_=d_bq[:].rearrange("(t p) -> p t", p=P))
            dma.dma_start(out=bks, in_=d_bk[:].rearrange("(t p) -> p t", p=P))
        nc.vector.memset(epsb, 1e-5)
        nc.gpsimd.memset(rmagic, 0x5F3759DF)
        # preload the exp ACT table while DMAs stream
        tdummy = misc.tile([1, 1], f32, tag="tdummy", name="tdummy")
        nc.scalar.activation(out=tdummy, in_=epsb[0:1, 0:1],
                             func=mybir.ActivationFunctionType.Exp)
        # chunked x loads, ordered so pair-0's first Q and K projections can
        # start as early as possible
        def load_xq_chunk(ch):
            csl = slice(ch * 512, (ch + 1) * 512)
            dma.dma_start(out=XQ[:, :, csl],
                          in_=d_xqT[:, csl].rearrange("(t p) k -> p t k", p=P))

        def load_xk_chunk(ch):
            csl = slice(ch * 512, (ch + 1) * 512)
            dma.dma_start(out=XK[:, :, csl],
                          in_=d_xkT[:, csl].rearrange("(t p) k -> p t k", p=P))

        load_xq_chunk(0)
        # first XK chunk rides the ACT engine's DMA queue, in parallel with
        # XQ chunk 0 on the sync queue
        nc.scalar.dma_start(
            out=XK[:, :, 0:512],
            in_=d_xkT[:, 0:512].rearrange("(t p) k -> p t k", p=P))
        load_xq_chunk(1)
        for ch in range(1, 4):
            load_xk_chunk(ch)
        dma.dma_start(out=WV, in_=d_wvT[:].rearrange("(t p) m -> p t m", p=P))
        if not skip_bias:
            dma.dma_start(out=bvb, in_=bcast_ap(d_bv))
        for g in range(KT // 2):
            v4 = VG[g].rearrange("p j (h w) -> p j h w", w=W65)
            nc.vector.memset(v4[:, :, :, D:W65], 1.0 / CTX_SCALE)
        if not skip_affine:
            dma.dma_start(out=lngb, in_=bcast_ap(d_lng))
            dma.dma_start(out=lnbb, in_=bcast_ap(d_lnb))
        dma.dma_start(out=WO, in_=d_woT[:].rearrange("(t p) m -> p t m", p=P))

        # ---- QK projection for one pair (DoubleRow over e-tile pairs) ----
        # PSUM->SBUF casts ride Pool to keep DVE/ACT free for exp work.
        def qk_emitters(p, wq_sl, wk_sl, qt, ktt):
            ems = []
            for ch in range(2):
                def eq(ch=ch):
                    csl = slice(ch * 512, (ch + 1) * 512)
                    ps = psum.tile([P, 512], f32, tag="acc", name="mmps")
                    for e2 in range(ET // 2):
                        nc.tensor.matmul(
                            ps, wq_sl[:, 2 * e2:2 * e2 + 2, :],
                            XQ[:, 2 * e2:2 * e2 + 2, csl],
                            start=(e2 == 0), stop=(e2 == ET // 2 - 1),
                            perf_mode=DR,
                        )
                    if skip_bias:
                        nc.scalar.activation(
                            out=qt[:, csl], in_=ps,
                            func=mybir.ActivationFunctionType.Identity)
                    else:
                        nc.vector.tensor_scalar_add(
                            out=qt[:, csl], in0=ps, scalar1=bqs[:, p:p + 1])
                ems.append(eq)
            for ch in range(4):
                def ek(ch=ch):
                    csl = slice(ch * 512, (ch + 1) * 512)
                    ps = psum.tile([P, 512], f32, tag="acc", name="mmps")
                    for e2 in range(ET // 2):
                        nc.tensor.matmul(
                            ps, wk_sl[:, 2 * e2:2 * e2 + 2, :],
                            XK[:, 2 * e2:2 * e2 + 2, csl],
                            start=(e2 == 0), stop=(e2 == ET // 2 - 1),
                            perf_mode=DR,
                        )
                    if skip_bias:
                        nc.scalar.activation(
                            out=ktt[:, csl], in_=ps,
                            func=mybir.ActivationFunctionType.Identity)
                    else:
                        nc.vector.tensor_scalar_add(
                            out=ktt[:, csl], in0=ps, scalar1=bks[:, p:p + 1])
                ems.append(ek)
            return ems

        def new_qk_tiles():
            qt = qkpool.tile([P, NQ], f8, tag="qtp", name="qtp")
            ktt = qkpool.tile([P, S], f8, tag="ktp", name="ktp")
            return qt, ktt

        def new_pt():
            return ppool.tile([P, 2, KT, 512], f8, tag="pt", name="pt")

        # ---- scores + exp/bit-trick for one k-tile, both heads ----
        def scores_one_group(qt, ktt, qc, g, pt, ci=0):
            qsl = slice(qc * 512, (qc + 1) * 512)
            ksl = slice(g * P, (g + 1) * P)
            sps = psum.tile([P, 2, 512], f32, tag="spsum", name="sps", bufs=3)
            for h in range(2):
                hsl = slice(h * D, (h + 1) * D)
                nc.tensor.matmul(
                    sps[:, h, :], ktt[hsl, ksl], qt[hsl, qsl],
                    start=True, stop=True,
                )
            out = pt[:, :, g, :]
            eng = _exp_assign(ci, g)
            if eng == "A":
                nc.scalar.activation(
                    out=out, in_=sps,
                    func=mybir.ActivationFunctionType.Exp,
                    scale=EXP_SCALE,
                )
            else:
                nc.vector.tensor_scalar_add(
                    out=out.bitcast(i8), in0=sps, scalar1=MAGIC)

        # ctx closures interleave into the NEXT chunk's scores: 2 DR pairs per
        # group slot (plus the norm chain), so the PE never emits ctx as a
        # blocking block and the exp engines keep a full sps backlog.
        CTX_POPS = [2, 2, 2, 2, 3, 2, 2, 2, 1, 0, 0, 0, 0, 0, 0, 0]

        def scores_exp(qt, ktt, qc, pt, ci=0, interleave=None, ctx_ems=None):
            for g in range(KT):
                scores_one_group(qt, ktt, qc, g, pt, ci=ci)
                if ctx_ems:
                    for _ in range(CTX_POPS[g]):
                        if ctx_ems:
                            ctx_ems.pop(0)()
                if interleave and g >= 10:
                    interleave.pop(0)()
            while ctx_ems:
                ctx_ems.pop(0)()

        # ---- ctx + normalize emitters for a previously exp'd chunk ----
        def ctx_emitters(p, qc, pt):
            qsl = slice(qc * 512, (qc + 1) * 512)
            cps = [None, None]
            ems = []
            for h in range(2):
                hh = 2 * p + h
                vsl = slice(hh * W65, (hh + 1) * W65)
                for j in range(KT // 2):
                    def epair(h=h, j=j, vsl=vsl):
                        if j == 0:
                            cps[h] = psum.tile([65, 512], f32, tag="acc",
                                               name=f"cps{h}")
                        nc.tensor.matmul(
                            cps[h], VG[j][:, 0:2, vsl],
                            pt[:, h, 2 * j:2 * j + 2, :],
                            start=(j == 0), stop=(j == KT // 2 - 1),
                            perf_mode=DR,
                        )
                    ems.append(epair)

                def enorm(h=h, hh=hh):
                    recip = misc.tile([1, 512], f32, tag="recip", name="recip",
                                      bufs=2)
                    nc.vector.reciprocal(out=recip, in_=cps[h][64:65, :])
                    bc = misc.tile([D, 512], f32, tag="bc", name="bc", bufs=2)
                    nc.gpsimd.partition_broadcast(out_ap=bc, in_ap=recip)
                    r0 = (hh % 2) * D
                    nc.vector.tensor_mul(
                        out=CTG[hh // 4][r0:r0 + D, (hh // 2) % 2, qsl],
                        in0=cps[h][0:D, :], in1=bc,
                    )
                ems.append(enorm)
            return ems

        def ctx_norm(p, qc, pt):
            for em in ctx_emitters(p, qc, pt):
                em()

        # ---- pair-0 projection with scores/exp interleaved per K chunk ----
        qk0 = new_qk_tiles()
        ems0 = qk_emitters(0, *w0, *qk0)
        ems0[0]()  # Q proj chunk 0
        ems0[1]()  # Q proj chunk 1
        pt00 = new_pt()
        for ch in range(4):
            ems0[2 + ch]()  # K proj chunk ch covers k-tiles 4ch..4ch+3
            for g in range(4 * ch, 4 * ch + 4):
                scores_one_group(*qk0, 0, g, pt00, ci=0)

        # ---- V projection (DoubleRow); pair-0 qc1 exps and pair-1 projection
        # both interleave under it so the exp engines never starve ----
        w1 = load_wslices(1)
        qk1 = new_qk_tiles()
        ems1 = qk_emitters(1, *w1, *qk1)
        pt01 = new_pt()
        gi = 0
        for g in range(KT // 2):
            for j in range(2):
                kt = 2 * g + j
                ksl = slice(kt * P, (kt + 1) * P)
                for ch in range(2):
                    csl = slice(ch * 512, (ch + 1) * 512)
                    ps = psum.tile([P, 512], f32, tag="acc", name="mmps")
                    for e2 in range(ET // 2):
                        nc.tensor.matmul(
                            ps, XK[:, 2 * e2:2 * e2 + 2, ksl],
                            WV[:, 2 * e2:2 * e2 + 2, csl],
                            start=(e2 == 0), stop=(e2 == ET // 2 - 1),
                            perf_mode=DR,
                        )
                    v4 = VG[g].rearrange("p j (h w) -> p j h w", w=W65)
                    if skip_bias:
                        nc.scalar.activation(
                            out=v4[:, j, ch * 8:(ch + 1) * 8, 0:D],
                            in_=ps.rearrange("p (h w) -> p h w", w=D),
                            func=mybir.ActivationFunctionType.Identity)
                    else:
                        nc.vector.tensor_add(
                            out=v4[:, j, ch * 8:(ch + 1) * 8, 0:D],
                            in0=ps.rearrange("p (h w) -> p h w", w=D),
                            in1=bvb[:, csl].rearrange("p (h w) -> p h w", w=D),
                        )
            # keep the exp engines fed during the V stretch with pair-0 qc1
            for _ in range(2):
                if gi < KT:
                    scores_one_group(*qk0, 1, gi, pt01, ci=1)
                    gi += 1
            if g >= 1 and ems1:
                ems1.pop(0)()

        # ---- pair-0 qc0 ctx, remaining pair-1 projection ----
        ctx_norm(0, 0, pt00)
        for em in ems1:
            em()

        # ---- out projection (DoubleRow) + residual + layernorm ----
        # residual DMA (d_xq already carries +bo, folded on host); LN stats via
        # fused accum_out sums (mean) + an ACT Square pass (E[x^2]) - no
        # bn_stats pass.  The elementwise normalize runs on Pool for the
        # mid-stream tiles (ACT/DVE are busy with exp) and on DVE at drain.
        xq_sbs = [None] * ET

        def load_xq(nt):
            xq_sb = xqp.tile([P, E], f32, tag="xqt", name="xqt")
            dma.dma_start(out=xq_sb, in_=d_xq[nt * P:(nt + 1) * P, :])
            xq_sbs[nt] = xq_sb

        for nt in range(4):
            load_xq(nt)

        def tail_nt(nt, drain=False):
            # prefetch the +4 residual tile; emitting it here (not upfront)
            # keeps the 4-buf ring's reuse dep behind this tile's reads
            if nt + 4 < ET:
                load_xq(nt + 4)
            nsl = slice(nt * P, (nt + 1) * P)
            xq_sb = xq_sbs[nt]
            out_sb = outp.tile([P, E], f32, tag="outsb", name="outsb")
            scratch = outp.tile([P, E], f32, tag="sq", name="sq", bufs=2)
            sq2 = misc.tile([P, 2], f32, tag="sq2", name="sq2")
            stat = misc.tile([P, 4], f32, tag="stat", name="stat")
            rsum = misc.tile([P, 2], f32, tag="rsum", name="rsum")
            for ec in range(2):
                csl = slice(ec * 512, (ec + 1) * 512)
                ps = psum.tile([P, 512], f32, tag="acc", name="mmps")
                for t in range(ET // 2):
                    nc.tensor.matmul(
                        ps, CTG[t][:, 0:2, nsl], WO[:, 2 * t:2 * t + 2, csl],
                        start=(t == 0), stop=(t == ET // 2 - 1),
                        perf_mode=DR,
                    )
                # out = ps/CTX_SCALE + (x + bo), accumulating row sums
                nc.vector.scalar_tensor_tensor(
                    out=out_sb[:, csl], in0=ps, scalar=1.0 / CTX_SCALE,
                    in1=xq_sb[:, csl],
                    op0=mybir.AluOpType.mult, op1=mybir.AluOpType.add,
                    accum_out=rsum[:, ec:ec + 1],
                )
                nc.scalar.activation(
                    out=scratch[:, csl], in_=out_sb[:, csl],
                    func=mybir.ActivationFunctionType.Square,
                    accum_out=sq2[:, ec:ec + 1])
            nc.gpsimd.tensor_add(out=stat[:, 0:1], in0=rsum[:, 0:1],
                                 in1=rsum[:, 1:2])
            # var*E = (sq0+sq1) - (r0+r1)^2/E; the final /E folds into Sqrt's
            # scale.  Small stats run on Pool (SBUF-only; no stt there).
            nc.gpsimd.tensor_tensor(
                out=stat[:, 1:2], in0=stat[:, 0:1], in1=stat[:, 0:1],
                op=mybir.AluOpType.mult)                             # msum^2
            nc.gpsimd.tensor_scalar(
                out=stat[:, 1:2], in0=stat[:, 1:2], scalar1=1.0 / E,
                scalar2=None, op0=mybir.AluOpType.mult)              # msum^2/E
            nc.gpsimd.tensor_add(out=stat[:, 2:3], in0=sq2[:, 0:1], in1=sq2[:, 1:2])
            nc.gpsimd.tensor_sub(out=stat[:, 3:4], in0=stat[:, 2:3], in1=stat[:, 1:2])
            # rstd via the 0x5f3759df rsqrt bit-trick + one Newton step, all
            # on Pool: avoids an ACT Sqrt (whose table set would thrash the
            # exp table mid-stream) and a DVE reciprocal.
            var = misc.tile([P, 1], f32, tag="var", name="var")
            nc.gpsimd.tensor_scalar(
                out=var, in0=stat[:, 3:4], scalar1=1.0 / E, scalar2=1e-5,
                op0=mybir.AluOpType.mult, op1=mybir.AluOpType.add)   # var+eps
            std = misc.tile([P, 1], f32, tag="std", name="std")
            istd = std.bitcast(mybir.dt.int32)
            nc.vector.tensor_scalar(
                out=istd, in0=var.bitcast(mybir.dt.int32), scalar1=1,
                scalar2=None, op0=mybir.AluOpType.arith_shift_right)
            nc.gpsimd.tensor_tensor(
                out=istd, in0=rmagic, in1=istd, op=mybir.AluOpType.subtract)
            # Newton: y *= 1.5 - 0.5*var*y^2
            nwt = misc.tile([P, 1], f32, tag="nwt", name="nwt")
            nc.gpsimd.tensor_tensor(out=nwt, in0=std, in1=std,
                                    op=mybir.AluOpType.mult)
            nc.gpsimd.tensor_tensor(out=nwt, in0=nwt, in1=var,
                                    op=mybir.AluOpType.mult)
            nc.gpsimd.tensor_scalar(
                out=nwt, in0=nwt, scalar1=-0.5, scalar2=1.5,
                op0=mybir.AluOpType.mult, op1=mybir.AluOpType.add)
            nc.gpsimd.tensor_tensor(out=std, in0=std, in1=nwt,
                                    op=mybir.AluOpType.mult)          # rstd
            negmr = misc.tile([P, 1], f32, tag="negmr", name="negmr")
            nc.gpsimd.tensor_scalar(
                out=negmr, in0=stat[:, 0:1], scalar1=-1.0 / E,
                scalar2=None, op0=mybir.AluOpType.mult)
            nc.gpsimd.tensor_tensor(
                out=negmr, in0=negmr, in1=std, op=mybir.AluOpType.mult)  # -mean*rstd
            # (x - mu) * rstd == x*rstd + (-mu*rstd)
            norm_eng = nc.vector if drain else nc.gpsimd
            norm_eng.tensor_scalar(
                out=out_sb, in0=out_sb,
                scalar1=std[:, 0:1], scalar2=negmr[:, 0:1],
                op0=mybir.AluOpType.mult, op1=mybir.AluOpType.add,
            )
            if not skip_affine:
                nc.vector.tensor_mul(out=out_sb, in0=out_sb, in1=lngb)
                nc.gpsimd.tensor_add(out=out_sb, in0=out_sb, in1=lnbb)
            dma.dma_start(out=d_out[nsl, :], in_=out_sb)

        # ---- pipelined attention, qc-MAJOR: sweep qc=0 over pairs 1..7
        # (staging each next pair's projections), so all qc=0 ctx completes
        # mid-kernel; the qc=1 sweep then interleaves tail_nt(0..3).  ctx for
        # chunk i runs one chunk behind its scores/exp. ----
        qks = [None, qk1] + [None] * (ET - 2)
        ctxq = [(0, 1, pt01)]   # (0,0) already ctx'd above
        tail_q = []
        ci = 2
        for sweep_qc in (0, 1):
            for p in range(1, ET):
                pending = []
                if sweep_qc == 0 and p + 1 < ET:
                    wnxt = load_wslices(p + 1)
                    qks[p + 1] = new_qk_tiles()
                    pending = qk_emitters(p + 1, *wnxt, *qks[p + 1])
                ptc = new_pt()
                done = ctxq.pop(0)
                cems = ctx_emitters(*done)
                scores_exp(*qks[p], sweep_qc, ptc, ci=ci, interleave=pending,
                           ctx_ems=cems)
                if done[:2] == (ET - 1, 0):
                    tail_q = list(range(4))
                for em in pending:
                    em()
                if tail_q and not pending:
                    tail_nt(tail_q.pop(0))
                ctxq.append((p, sweep_qc, ptc))
                ci += 1

        ctx_norm(*ctxq.pop(0))
        for nt in tail_q + list(range(4, ET)):
            tail_nt(nt, drain=True)

    nc.compile()
    return nc


def _get_nc(skip_affine=False, skip_bias=False):
    key = ("nc", skip_affine, skip_bias)
    if key not in _cache:
        _cache[key] = _build_nc(skip_affine, skip_bias)
    return _cache[key]


def kernel(x, Wq, bq, Wk, bk, Wv, bv, Wo, bo, ln_g, ln_b, _trace=False, _tmpdir=None):
    from concourse.bass_utils import run_bass_kernel_spmd

    x = np.asarray(x, np.float32)

    def shuffle_w(W, scale=1.0):
        # W.T [e_in, e_out] -> [pair, p, t, m] with e_in = t*128+p, e_out of pair
        wT = (np.asarray(W, np.float32) * scale).T.reshape(ET, P, ET, P)
        return np.ascontiguousarray(wT.transpose(2, 1, 0, 3)).astype(FP8)

    wqR = shuffle_w(Wq)
    # Wk pre-scaled by 1/ln2: PSUM scores become s/ln2, which both the ACT
    # exp (scale=0.125*ln2) and the fp8-bits trick (int8(p + 55.5)) consume.
    wkR = shuffle_w(Wk, scale=INV_LN2)
    wvT = np.ascontiguousarray(np.asarray(Wv, np.float32).T).astype(FP8)
    woT = np.ascontiguousarray(np.asarray(Wo, np.float32).T).astype(FP8)
    vecs = {
        "bq": np.asarray(bq, np.float32),
        "bk": np.asarray(bk, np.float32) * np.float32(INV_LN2),
        "bv": np.asarray(bv, np.float32),
        "lng": np.asarray(ln_g, np.float32), "lnb": np.asarray(ln_b, np.float32),
    }
    bo_f = np.asarray(bo, np.float32)

    in_maps = []
    for c in range(NCORES):
        b, half = c // 2, c % 2
        xbT = np.ascontiguousarray(x[b].T).astype(FP8)
        xq_res = x[b, half * NQ:(half + 1) * NQ, :] + bo_f
        in_maps.append({
            "xkT": xbT,
            "xqT": np.ascontiguousarray(xbT[:, half * NQ:(half + 1) * NQ]),
            # residual rows with the out-proj bias pre-added (saves a DVE pass)
            "xq": xq_res,
            "wqR": wqR, "wkR": wkR, "wvT": wvT, "woT": woT,
            **vecs,
        })

    # ln_g == 1 / ln_b == 0 make the LN affine step an exact no-op; build the
    # specialized kernel for that case (general path kept as fallback)
    skip_affine = bool(
        np.all(np.asarray(ln_g) == 1.0) and np.all(np.asarray(ln_b) == 0.0))
    skip_bias = bool(
        np.all(np.asarray(bq) == 0.0) and np.all(np.asarray(bk) == 0.0)
        and np.all(np.asarray(bv) == 0.0))
    if skip_bias:
        for m in in_maps:
            del m["bq"], m["bk"], m["bv"]
    nc = _get_nc(skip_affine, skip_bias)
    _cache["last_nc"] = nc
    res = run_bass_kernel_spmd(
        nc, in_maps, list(range(NCORES)), trace=_trace, tmpdir=_tmpdir
    )
    out = np.empty((B, S, E), np.float32)
    for c in range(NCORES):
        b, half = c // 2, c % 2
        out[b, half * NQ:(half + 1) * NQ, :] = res.results[c]["out"]
    if _trace:
        _cache["last_result"] = res
    return out


# revision 68
# speedup vs baseline: 1.0069x; 1.0064x over previous
"""Trainium2 Bass kernel for a dense transformer attention block.

Reference (per batch b of 4, seq S=2048, embed E=1024, H=16 heads, D=64):
    q/k/v = x @ W{q,k,v}.T + b,  split heads
    attn  = softmax(q k^T / sqrt(D)),  ctx = attn @ v
    out   = LN(ctx @ Wo.T + bo + x) * ln_g + ln_b

Sharding (8 cores, no collectives): core c handles batch b=c//2 and query rows
[1024*(c%2), 1024*(c%2+1)).  Each core computes K/V projections for its full
batch (duplicated with its pair core, ~25% extra FLOPs, zero comms), attention
for all 16 heads over its 1024 query rows, out-projection + residual + LN for
its rows.  Host reassembles the 8 row-shards.

Core layout strategy:
  - scores computed TRANSPOSED: S^T[k, q] with K^T stationary / Q^T moving, so
    exp(S^T) feeds the ctx matmul directly as the moving operand - no PE
    transposes anywhere in the kernel.
  - softmax exp is SPLIT ACROSS THREE ENGINES.  Wk is pre-scaled by 1/ln2 on
    the host so PSUM scores hold p = s/ln2 = 8*log2(exp(s/8)).  ACT computes
    table exp (scale=0.125*ln2) for some k-tiles; Pool and DVE compute the
    fp8e4m3 BIT PATTERN of 2^(p/8) directly as int8(p + 55.5) (one
    tensor_scalar_add with round-to-nearest convert; fp8 bits of 2^u are
    8u+56, so the linear mantissa interp gives exp to ~3% rms - below the
    fp8 quantization the ACT path pays anyway).  The int8 result is bitcast
    to fp8e4 and fed straight to the ctx matmul.
  - per k-tile group, both heads' scores live in one [P, 2, 512] PSUM tile
    (2 banks, ring of 3), and exp output lands in a per-chunk persistent
    pt[P, 2 heads, 16 kt, 512] fp8 tile so ALL ctx matmuls run as clean
    even-aligned DoubleRow pairs (0.5 cyc/row).
  - chunks run qc-MAJOR (all pairs at q-half 0, then q-half 1) so the first
    half of the out-proj/LN tail interleaves into the second sweep; ctx DR
    pairs of chunk i-1 interleave INTO chunk i's score stream (CTX_POPS);
    the LN rstd uses the 0x5f3759df rsqrt bit-trick + one Newton step on
    Pool (no ACT table swap mid-stream, no DVE reciprocal).
  - softmax denominator from a ones-column appended to V (stationary [V_h|1],
    M=65): PSUM row 64 accumulates sum_k exp.  The ones column holds 1/16 so
    the reciprocal is directly 16/sum (CTX_SCALE=16 keeps fp8 ctx out of the
    subnormal range; /16 folded into the out-proj epilogue).
  - ctx lands as ctx^T[e, q], exactly the stationary layout out-proj needs.
  - q/k/v/out projections and ctx run in fp8e4 with DoubleRow (0.5 cyc/row);
    PSUM accumulation is always fp32, residual + layernorm in fp32.  The
    attention branch contributes only ~0.8% of the output magnitude (the
    residual dominates), so fp8 path error dilutes ~128x in the final output.
  - scores+exp for chunk i overlap ctx for chunk i-1 (software pipeline), and
    the next pair's projection matmuls are interleaved into the attention
    stream so the in-order PE queue always has work.
"""

import sys

if "/opt/trn_rl_repo" not in sys.path:
    sys.path.insert(0, "/opt/trn_rl_repo")

import numpy as np
import ml_dtypes

B, S, E = 4, 2048, 1024
H, D = 16, 64
NQ = S // 2          # query rows per core
P = 128
ET = E // P          # 8 e-tiles
KT = S // P          # 16 k-tiles
W65 = D + 1          # V head width incl. ones column
VW = H * W65         # 1040
NCORES = 8
CTX_SCALE = 16.0     # keep fp8 ctx in normal range

INV_LN2 = 1.4426950408889634
EXP_SCALE = 0.125 / INV_LN2     # ACT exp scale after Wk prescale
MAGIC = 55.5                    # fp8e4m3 exponent bias*8 - 0.5 (round-nearest)

# per-chunk k-tile -> exp engine: A=ACT table exp, V=DVE bit-trick.
# (Pool/GPSIMD cannot access PSUM.)  The ACT/DVE split is phase-aware:
# during startup (ci=0) and the V-projection stretch (ci=1) ACT is loaded
# with PSUM casts while DVE would otherwise idle, so DVE takes nearly all
# exp groups there; the qc0 sweep carries projection casts on ACT (a=8);
# the qc1 sweep has no casts so ACT takes more (a=10).
_A_COUNT = {0: 4, 1: 0}


def _exp_assign(ci, g):
    a = _A_COUNT.get(ci, 8 if ci <= 8 else 10)
    return "A" if ((g + 1) * a) // KT > (g * a) // KT else "V"

FP8 = ml_dtypes.float8_e4m3

_cache = {}


def _build_nc(skip_affine=False, skip_bias=False):
    import concourse.bass as bass
    import concourse.tile as tile
    from concourse import bacc, mybir

    f8 = mybir.dt.float8e4
    i8 = mybir.dt.int8
    f32 = mybir.dt.float32
    DR = mybir.MatmulPerfMode.DoubleRow

    nc = bacc.Bacc(None, target_bir_lowering=False, debug=False)

    d_xkT = nc.dram_tensor("xkT", [E, S], f8, kind="ExternalInput")
    d_xqT = nc.dram_tensor("xqT", [E, NQ], f8, kind="ExternalInput")
    d_xq = nc.dram_tensor("xq", [NQ, E], f32, kind="ExternalInput")
    # wq/wk pre-shuffled on host to [pair, p, t, m] so each pair's slice DMAs
    # as contiguous 1KB runs instead of 128B strided rows
    d_wqR = nc.dram_tensor("wqR", [ET, P, ET, P], f8, kind="ExternalInput")
    d_wkR = nc.dram_tensor("wkR", [ET, P, ET, P], f8, kind="ExternalInput")
    d_wvT = nc.dram_tensor("wvT", [E, E], f8, kind="ExternalInput")
    d_woT = nc.dram_tensor("woT", [E, E], f8, kind="ExternalInput")
    if not skip_bias:
        d_bq = nc.dram_tensor("bq", [E], f32, kind="ExternalInput")
        d_bk = nc.dram_tensor("bk", [E], f32, kind="ExternalInput")
        d_bv = nc.dram_tensor("bv", [E], f32, kind="ExternalInput")
    d_lng = nc.dram_tensor("lng", [E], f32, kind="ExternalInput")
    d_lnb = nc.dram_tensor("lnb", [E], f32, kind="ExternalInput")
    d_out = nc.dram_tensor("out", [NQ, E], f32, kind="ExternalOutput")

    def bcast_ap(d):
        ap = d[:]
        return bass.AP(tensor=ap.tensor, offset=ap.offset, ap=[[0, P], [1, E]])

    from contextlib import ExitStack

    with tile.TileContext(nc) as tc, ExitStack() as ctx:
        persist = ctx.enter_context(tc.tile_pool(name="persist", bufs=1))
        wslice = ctx.enter_context(tc.tile_pool(name="wslice", bufs=2))
        # qc-major chunk order keeps every pair's q/k tiles alive
        qkpool = ctx.enter_context(tc.tile_pool(name="qkpool", bufs=ET))
        ppool = ctx.enter_context(tc.tile_pool(name="ppool", bufs=3))
        misc = ctx.enter_context(tc.tile_pool(name="misc", bufs=4))
        xqp = ctx.enter_context(tc.tile_pool(name="xqp", bufs=4))
        outp = ctx.enter_context(tc.tile_pool(name="outp", bufs=4))
        psum = ctx.enter_context(tc.tile_pool(name="psum", bufs=2, space="PSUM"))

        dma = nc.sync

        # ---- persistent tiles ----
        XK = persist.tile([P, ET, S], f8, tag="XK")       # x[b]^T, e-tiles on dim1
        XQ = persist.tile([P, ET, NQ], f8, tag="XQ")      # my query rows ^T
        WV = persist.tile([P, ET, E], f8, tag="WV")
        WO = persist.tile([P, ET, E], f8, tag="WO")
        VG = [persist.tile([P, 2, VW], f8, tag=f"vg{g}", name=f"vg{g}")
              for g in range(KT // 2)]
        CTG = [persist.tile([P, 2, NQ], f8, tag=f"ctg{t}", name=f"ctg{t}")
               for t in range(ET // 2)]
        if not skip_bias:
            bqs = persist.tile([P, ET], f32, tag="bqs")
            bks = persist.tile([P, ET], f32, tag="bks")
            bvb = persist.tile([P, E], f32, tag="bvb")
        if not skip_affine:
            lngb = persist.tile([P, E], f32, tag="lngb")
            lnbb = persist.tile([P, E], f32, tag="lnbb")
        epsb = persist.tile([P, 1], f32, tag="epsb")
        # rsqrt magic constant 0x5f3759df as int32, for the tail bit-trick
        rmagic = persist.tile([P, 1], mybir.dt.int32, tag="rmagic")

        # ---- input loads, ordered by first use ----
        def load_wslices(p):
            wq_sl = wslice.tile([P, ET, P], f8, tag="wqsl", name="wqsl")
            wk_sl = wslice.tile([P, ET, P], f8, tag="wksl", name="wksl")
            nc.gpsimd.dma_start(out=wq_sl, in_=d_wqR[p])
            nc.gpsimd.dma_start(out=wk_sl, in_=d_wkR[p])
            return wq_sl, wk_sl

        w0 = load_wslices(0)
        if not skip_bias:
            dma.dma_start(out=bqs, in_=d_bq[:].rearrange("(t p) -> p t", p=P))
            dma.dma_start(out=bks, in_=d_bk[:].rearrange("(t p) -> p t", p=P))
        nc.vector.memset(epsb, 1e-5)
        nc.gpsimd.memset(rmagic, 0x5F3759DF)
        # preload the exp ACT table while DMAs stream
        tdummy = misc.tile([1, 1], f32, tag="tdummy", name="tdummy")
        nc.scalar.activation(out=tdummy, in_=epsb[0:1, 0:1],
                             func=mybir.ActivationFunctionType.Exp)
        # chunked x loads, ordered so pair-0's first Q and K projections can
        # start as early as possible
        def load_xq_chunk(ch):
            csl = slice(ch * 512, (ch + 1) * 512)
            dma.dma_start(out=XQ[:, :, csl],
                          in_=d_xqT[:, csl].rearrange("(t p) k -> p t k", p=P))

        def load_xk_chunk(ch):
            csl = slice(ch * 512, (ch + 1) * 512)
            dma.dma_start(out=XK[:, :, csl],
                          in_=d_xkT[:, csl].rearrange("(t p) k -> p t k", p=P))

        load_xq_chunk(0)
        # first XK chunk rides the ACT engine's DMA queue, in parallel with
        # XQ chunk 0 on the sync queue
        nc.scalar.dma_start(
            out=XK[:, :, 0:512],
            in_=d_xkT[:, 0:512].rearrange("(t p) k -> p t k", p=P))
        load_xq_chunk(1)
        for ch in range(1, 4):
            load_xk_chunk(ch)
        dma.dma_start(out=WV, in_=d_wvT[:].rearrange("(t p) m -> p t m", p=P))
        if not skip_bias:
            dma.dma_start(out=bvb, in_=bcast_ap(d_bv))
        for g in range(KT // 2):
            v4 = VG[g].rearrange("p j (h w) -> p j h w", w=W65)
            nc.vector.memset(v4[:, :, :, D:W65], 1.0 / CTX_SCALE)
        if not skip_affine:
            dma.dma_start(out=lngb, in_=bcast_ap(d_lng))
            dma.dma_start(out=lnbb, in_=bcast_ap(d_lnb))
        dma.dma_start(out=WO, in_=d_woT[:].rearrange("(t p) m -> p t m", p=P))

        # ---- QK projection for one pair (DoubleRow over e-tile pairs) ----
        # PSUM->SBUF casts ride Pool to keep DVE/ACT free for exp work.
        def qk_emitters(p, wq_sl, wk_sl, qt, ktt):
            ems = []
            for ch in range(2):
                def eq(ch=ch):
                    csl = slice(ch * 512, (ch + 1) * 512)
                    ps = psum.tile([P, 512], f32, tag="acc", name="mmps")
                    for e2 in range(ET // 2):
                        nc.tensor.matmul(
                            ps, wq_sl[:, 2 * e2:2 * e2 + 2, :],
                            XQ[:, 2 * e2:2 * e2 + 2, csl],
                            start=(e2 == 0), stop=(e2 == ET // 2 - 1),
                            perf_mode=DR,
                        )
                    if skip_bias:
                        nc.scalar.activation(
                            out=qt[:, csl], in_=ps,
                            func=mybir.ActivationFunctionType.Identity)
                    else:
                        nc.vector.tensor_scalar_add(
                            out=qt[:, csl], in0=ps, scalar1=bqs[:, p:p + 1])
                ems.append(eq)
            for ch in range(4):
                def ek(ch=ch):
                    csl = slice(ch * 512, (ch + 1) * 512)
                    ps = psum.tile([P, 512], f32, tag="acc", name="mmps")
                    for e2 in range(ET // 2):
                        nc.tensor.matmul(
                            ps, wk_sl[:, 2 * e2:2 * e2 + 2, :],
                            XK[:, 2 * e2:2 * e2 + 2, csl],
                            start=(e2 == 0), stop=(e2 == ET // 2 - 1),
                            perf_mode=DR,
                        )
                    if skip_bias:
                        nc.scalar.activation(
                            out=ktt[:, csl], in_=ps,
                            func=mybir.ActivationFunctionType.Identity)
                    else:
                        nc.vector.tensor_scalar_add(
                            out=ktt[:, csl], in0=ps, scalar1=bks[:, p:p + 1])
                ems.append(ek)
            return ems

        def new_qk_tiles():
            qt = qkpool.tile([P, NQ], f8, tag="qtp", name="qtp")
            ktt = qkpool.tile([P, S], f8, tag="ktp", name="ktp")
            return qt, ktt

        def new_pt():
            return ppool.tile([P, 2, KT, 512], f8, tag="pt", name="pt")

        # ---- scores + exp/bit-trick for one k-tile, both heads ----
        def scores_one_group(qt, ktt, qc, g, pt, ci=0):
            qsl = slice(qc * 512, (qc + 1) * 512)
            ksl = slice(g * P, (g + 1) * P)
            sps = psum.tile([P, 2, 512], f32, tag="spsum", name="sps", bufs=3)
            for h in range(2):
                hsl = slice(h * D, (h + 1) * D)
                nc.tensor.matmul(
                    sps[:, h, :], ktt[hsl, ksl], qt[hsl, qsl],
                    start=True, stop=True,
                )
            out = pt[:, :, g, :]
            eng = _exp_assign(ci, g)
            if eng == "A":
                nc.scalar.activation(
                    out=out, in_=sps,
                    func=mybir.ActivationFunctionType.Exp,
                    scale=EXP_SCALE,
                )
            else:
                nc.vector.tensor_scalar_add(
                    out=out.bitcast(i8), in0=sps, scalar1=MAGIC)

        # ctx closures interleave into the NEXT chunk's scores: 2 DR pairs per
        # group slot (plus the norm chain), so the PE never emits ctx as a
        # blocking block and the exp engines keep a full sps backlog.
        CTX_POPS = [2, 2, 2, 2, 3, 2, 2, 2, 1, 0, 0, 0, 0, 0, 0, 0]

        def scores_exp(qt, ktt, qc, pt, ci=0, interleave=None, ctx_ems=None):
            for g in range(KT):
                scores_one_group(qt, ktt, qc, g, pt, ci=ci)
                if ctx_ems:
                    for _ in range(CTX_POPS[g]):
                        if ctx_ems:
                            ctx_ems.pop(0)()
                if interleave and g >= 9:
                    interleave.pop(0)()
            while ctx_ems:
                ctx_ems.pop(0)()

        # ---- ctx + normalize emitters for a previously exp'd chunk ----
        def ctx_emitters(p, qc, pt):
            qsl = slice(qc * 512, (qc + 1) * 512)
            cps = [None, None]
            ems = []
            for h in range(2):
                hh = 2 * p + h
                vsl = slice(hh * W65, (hh + 1) * W65)
                for j in range(KT // 2):
                    def epair(h=h, j=j, vsl=vsl):
                        if j == 0:
                            cps[h] = psum.tile([65, 512], f32, tag="acc",
                                               name=f"cps{h}")
                        nc.tensor.matmul(
                            cps[h], VG[j][:, 0:2, vsl],
                            pt[:, h, 2 * j:2 * j + 2, :],
                            start=(j == 0), stop=(j == KT // 2 - 1),
                            perf_mode=DR,
                        )
                    ems.append(epair)

                def enorm(h=h, hh=hh):
                    recip = misc.tile([1, 512], f32, tag="recip", name="recip",
                                      bufs=2)
                    nc.vector.reciprocal(out=recip, in_=cps[h][64:65, :])
                    bc = misc.tile([D, 512], f32, tag="bc", name="bc", bufs=2)
                    nc.gpsimd.partition_broadcast(out_ap=bc, in_ap=recip)
                    r0 = (hh % 2) * D
                    nc.vector.tensor_mul(
                        out=CTG[hh // 4][r0:r0 + D, (hh // 2) % 2, qsl],
                        in0=cps[h][0:D, :], in1=bc,
                    )
                ems.append(enorm)
            return ems

        def ctx_norm(p, qc, pt):
            for em in ctx_emitters(p, qc, pt):
                em()

        # ---- pair-0 projection with scores/exp interleaved per K chunk ----
        qk0 = new_qk_tiles()
        ems0 = qk_emitters(0, *w0, *qk0)
        ems0[0]()  # Q proj chunk 0
        ems0[1]()  # Q proj chunk 1
        pt00 = new_pt()
        for ch in range(4):
            ems0[2 + ch]()  # K proj chunk ch covers k-tiles 4ch..4ch+3
            for g in range(4 * ch, 4 * ch + 4):
                scores_one_group(*qk0, 0, g, pt00, ci=0)

        # ---- V projection (DoubleRow); pair-0 qc1 exps and pair-1 projection
        # both interleave under it so the exp engines never starve ----
        w1 = load_wslices(1)
        qk1 = new_qk_tiles()
        ems1 = qk_emitters(1, *w1, *qk1)
        pt01 = new_pt()
        gi = 0
        for g in range(KT // 2):
            for j in range(2):
                kt = 2 * g + j
                ksl = slice(kt * P, (kt + 1) * P)
                for ch in range(2):
                    csl = slice(ch * 512, (ch + 1) * 512)
                    ps = psum.tile([P, 512], f32, tag="acc", name="mmps")
                    for e2 in range(ET // 2):
                        nc.tensor.matmul(
                            ps, XK[:, 2 * e2:2 * e2 + 2, ksl],
                            WV[:, 2 * e2:2 * e2 + 2, csl],
                            start=(e2 == 0), stop=(e2 == ET // 2 - 1),
                            perf_mode=DR,
                        )
                    v4 = VG[g].rearrange("p j (h w) -> p j h w", w=W65)
                    if skip_bias:
                        nc.scalar.activation(
                            out=v4[:, j, ch * 8:(ch + 1) * 8, 0:D],
                            in_=ps.rearrange("p (h w) -> p h w", w=D),
                            func=mybir.ActivationFunctionType.Identity)
                    else:
                        nc.vector.tensor_add(
                            out=v4[:, j, ch * 8:(ch + 1) * 8, 0:D],
                            in0=ps.rearrange("p (h w) -> p h w", w=D),
                            in1=bvb[:, csl].rearrange("p (h w) -> p h w", w=D),
                        )
            # keep the exp engines fed during the V stretch with pair-0 qc1
            for _ in range(2):
                if gi < KT:
                    scores_one_group(*qk0, 1, gi, pt01, ci=1)
                    gi += 1
            if g >= 1 and ems1:
                ems1.pop(0)()

        # ---- pair-0 qc0 ctx, remaining pair-1 projection ----
        ctx_norm(0, 0, pt00)
        for em in ems1:
            em()

        # ---- out projection (DoubleRow) + residual + layernorm ----
        # residual DMA (d_xq already carries +bo, folded on host); LN stats via
        # fused accum_out sums (mean) + an ACT Square pass (E[x^2]) - no
        # bn_stats pass.  The elementwise normalize runs on Pool for the
        # mid-stream tiles (ACT/DVE are busy with exp) and on DVE at drain.
        xq_sbs = [None] * ET

        def load_xq(nt):
            xq_sb = xqp.tile([P, E], f32, tag="xqt", name="xqt")
            dma.dma_start(out=xq_sb, in_=d_xq[nt * P:(nt + 1) * P, :])
            xq_sbs[nt] = xq_sb

        for nt in range(4):
            load_xq(nt)

        def tail_nt(nt, drain=False):
            # prefetch the +4 residual tile; emitting it here (not upfront)
            # keeps the 4-buf ring's reuse dep behind this tile's reads
            if nt + 4 < ET:
                load_xq(nt + 4)
            nsl = slice(nt * P, (nt + 1) * P)
            xq_sb = xq_sbs[nt]
            out_sb = outp.tile([P, E], f32, tag="outsb", name="outsb")
            scratch = outp.tile([P, E], f32, tag="sq", name="sq", bufs=2)
            sq2 = misc.tile([P, 2], f32, tag="sq2", name="sq2")
            stat = misc.tile([P, 4], f32, tag="stat", name="stat")
            rsum = misc.tile([P, 2], f32, tag="rsum", name="rsum")
            for ec in range(2):
                csl = slice(ec * 512, (ec + 1) * 512)
                ps = psum.tile([P, 512], f32, tag="acc", name="mmps")
                for t in range(ET // 2):
                    nc.tensor.matmul(
                        ps, CTG[t][:, 0:2, nsl], WO[:, 2 * t:2 * t + 2, csl],
                        start=(t == 0), stop=(t == ET // 2 - 1),
                        perf_mode=DR,
                    )
                # out = ps/CTX_SCALE + (x + bo), accumulating row sums
                nc.vector.scalar_tensor_tensor(
                    out=out_sb[:, csl], in0=ps, scalar=1.0 / CTX_SCALE,
                    in1=xq_sb[:, csl],
                    op0=mybir.AluOpType.mult, op1=mybir.AluOpType.add,
                    accum_out=rsum[:, ec:ec + 1],
                )
                nc.scalar.activation(
                    out=scratch[:, csl], in_=out_sb[:, csl],
                    func=mybir.ActivationFunctionType.Square,
                    accum_out=sq2[:, ec:ec + 1])
            nc.gpsimd.tensor_add(out=stat[:, 0:1], in0=rsum[:, 0:1],
                                 in1=rsum[:, 1:2])
            # var*E = (sq0+sq1) - (r0+r1)^2/E; the final /E folds into Sqrt's
            # scale.  Small stats run on Pool (SBUF-only; no stt there).
            nc.gpsimd.tensor_tensor(
                out=stat[:, 1:2], in0=stat[:, 0:1], in1=stat[:, 0:1],
                op=mybir.AluOpType.mult)                             # msum^2
            nc.gpsimd.tensor_scalar(
                out=stat[:, 1:2], in0=stat[:, 1:2], scalar1=1.0 / E,
                scalar2=None, op0=mybir.AluOpType.mult)              # msum^2/E
            nc.gpsimd.tensor_add(out=stat[:, 2:3], in0=sq2[:, 0:1], in1=sq2[:, 1:2])
            nc.gpsimd.tensor_sub(out=stat[:, 3:4], in0=stat[:, 2:3], in1=stat[:, 1:2])
            # rstd via the 0x5f3759df rsqrt bit-trick + one Newton step, all
            # on Pool: avoids an ACT Sqrt (whose table set would thrash the
            # exp table mid-stream) and a DVE reciprocal.
            var = misc.tile([P, 1], f32, tag="var", name="var")
            nc.gpsimd.tensor_scalar(
                out=var, in0=stat[:, 3:4], scalar1=1.0 / E, scalar2=1e-5,
                op0=mybir.AluOpType.mult, op1=mybir.AluOpType.add)   # var+eps
            std = misc.tile([P, 1], f32, tag="std", name="std")
            istd = std.bitcast(mybir.dt.int32)
            nc.vector.tensor_scalar(
                out=istd, in0=var.bitcast(mybir.dt.int32), scalar1=1,
                scalar2=None, op0=mybir.AluOpType.arith_shift_right)
            nc.gpsimd.tensor_tensor(
                out=istd, in0=rmagic, in1=istd, op=mybir.AluOpType.subtract)
            # Newton: y *= 1.5 - 0.5*var*y^2
            nwt = misc.tile([P, 1], f32, tag="nwt", name="nwt")
            nc.gpsimd.tensor_tensor(out=nwt, in0=std, in1=std,
                                    op=mybir.AluOpType.mult)
            nc.gpsimd.tensor_tensor(out=nwt, in0=nwt, in1=var,
                                    op=mybir.AluOpType.mult)
            nc.gpsimd.tensor_scalar(
                out=nwt, in0=nwt, scalar1=-0.5, scalar2=1.5,
                op0=mybir.AluOpType.mult, op1=mybir.AluOpType.add)
            nc.gpsimd.tensor_tensor(out=std, in0=std, in1=nwt,
                                    op=mybir.AluOpType.mult)          # rstd
            negmr = misc.tile([P, 1], f32, tag="negmr", name="negmr")
            nc.gpsimd.tensor_scalar(
                out=negmr, in0=stat[:, 0:1], scalar1=-1.0 / E,
                scalar2=None, op0=mybir.AluOpType.mult)
            nc.gpsimd.tensor_tensor(
                out=negmr, in0=negmr, in1=std, op=mybir.AluOpType.mult)  # -mean*rstd
            # (x - mu) * rstd == x*rstd + (-mu*rstd)
            norm_eng = nc.vector if drain else nc.gpsimd
            norm_eng.tensor_scalar(
                out=out_sb, in0=out_sb,
                scalar1=std[:, 0:1], scalar2=negmr[:, 0:1],
                op0=mybir.AluOpType.mult, op1=mybir.AluOpType.add,
            )
            if not skip_affine:
                nc.vector.tensor_mul(out=out_sb, in0=out_sb, in1=lngb)
                nc.gpsimd.tensor_add(out=out_sb, in0=out_sb, in1=lnbb)
            dma.dma_start(out=d_out[nsl, :], in_=out_sb)

        # ---- pipelined attention, qc-MAJOR: sweep qc=0 over pairs 1..7
        # (staging each next pair's projections), so all qc=0 ctx completes
        # mid-kernel; the qc=1 sweep then interleaves tail_nt(0..3).  ctx for
        # chunk i runs one chunk behind its scores/exp. ----
        qks = [None, qk1] + [None] * (ET - 2)
        ctxq = [(0, 1, pt01)]   # (0,0) already ctx'd above
        tail_q = []
        ci = 2
        for sweep_qc in (0, 1):
            for p in range(1, ET):
                pending = []
                if sweep_qc == 0 and p + 1 < ET:
                    wnxt = load_wslices(p + 1)
                    qks[p + 1] = new_qk_tiles()
                    pending = qk_emitters(p + 1, *wnxt, *qks[p + 1])
                ptc = new_pt()
                done = ctxq.pop(0)
                cems = ctx_emitters(*done)
                scores_exp(*qks[p], sweep_qc, ptc, ci=ci, interleave=pending,
                           ctx_ems=cems)
                if done[:2] == (ET - 1, 0):
                    tail_q = list(range(4))
                for em in pending:
                    em()
                if tail_q and not pending:
                    tail_nt(tail_q.pop(0))
                ctxq.append((p, sweep_qc, ptc))
                ci += 1

        ctx_norm(*ctxq.pop(0))
        for nt in tail_q + list(range(4, ET)):
            tail_nt(nt, drain=True)

    nc.compile()
    return nc


def _get_nc(skip_affine=False, skip_bias=False):
    key = ("nc", skip_affine, skip_bias)
    if key not in _cache:
        _cache[key] = _build_nc(skip_affine, skip_bias)
    return _cache[key]


def kernel(x, Wq, bq, Wk, bk, Wv, bv, Wo, bo, ln_g, ln_b, _trace=False, _tmpdir=None):
    from concourse.bass_utils import run_bass_kernel_spmd

    x = np.asarray(x, np.float32)

    def shuffle_w(W, scale=1.0):
        # W.T [e_in, e_out] -> [pair, p, t, m] with e_in = t*128+p, e_out of pair
        wT = (np.asarray(W, np.float32) * scale).T.reshape(ET, P, ET, P)
        return np.ascontiguousarray(wT.transpose(2, 1, 0, 3)).astype(FP8)

    wqR = shuffle_w(Wq)
    # Wk pre-scaled by 1/ln2: PSUM scores become s/ln2, which both the ACT
    # exp (scale=0.125*ln2) and the fp8-bits trick (int8(p + 55.5)) consume.
    wkR = shuffle_w(Wk, scale=INV_LN2)
    wvT = np.ascontiguousarray(np.asarray(Wv, np.float32).T).astype(FP8)
    woT = np.ascontiguousarray(np.asarray(Wo, np.float32).T).astype(FP8)
    vecs = {
        "bq": np.asarray(bq, np.float32),
        "bk": np.asarray(bk, np.float32) * np.float32(INV_LN2),
        "bv": np.asarray(bv, np.float32),
        "lng": np.asarray(ln_g, np.float32), "lnb": np.asarray(ln_b, np.float32),
    }
    bo_f = np.asarray(bo, np.float32)

    in_maps = []
    for c in range(NCORES):
        b, half = c // 2, c % 2
        xbT = np.ascontiguousarray(x[b].T).astype(FP8)
        xq_res = x[b, half * NQ:(half + 1) * NQ, :] + bo_f
        in_maps.append({
            "xkT": xbT,
            "xqT": np.ascontiguousarray(xbT[:, half * NQ:(half + 1) * NQ]),
            # residual rows with the out-proj bias pre-added (saves a DVE pass)
            "xq": xq_res,
            "wqR": wqR, "wkR": wkR, "wvT": wvT, "woT": woT,
            **vecs,
        })

    # ln_g == 1 / ln_b == 0 make the LN affine step an exact no-op; build the
    # specialized kernel for that case (general path kept as fallback)
    skip_affine = bool(
        np.all(np.asarray(ln_g) == 1.0) and np.all(np.asarray(ln_b) == 0.0))
    skip_bias = bool(
        np.all(np.asarray(bq) == 0.0) and np.all(np.asarray(bk) == 0.0)
        and np.all(np.asarray(bv) == 0.0))
    if skip_bias:
        for m in in_maps:
            del m["bq"], m["bk"], m["bv"]
    nc = _get_nc(skip_affine, skip_bias)
    _cache["last_nc"] = nc
    res = run_bass_kernel_spmd(
        nc, in_maps, list(range(NCORES)), trace=_trace, tmpdir=_tmpdir
    )
    out = np.empty((B, S, E), np.float32)
    for c in range(NCORES):
        b, half = c // 2, c % 2
        out[b, half * NQ:(half + 1) * NQ, :] = res.results[c]["out"]
    if _trace:
        _cache["last_result"] = res
    return out
